# revision 1
# baseline (speedup 1.0000x reference)
"""Trainium2 Bass kernel for BaselineBlockNetSingleGraph (GRU + attention-GCN + convs + big linear).

Sharding: data-parallel over batch B=64 across 8 cores (8 batches/core) for
everything up to the final linear; the final linear's 196608-wide reduction is
column-sharded across cores (24576 each) via an on-device AllToAll of the
activations (bf16), with an AllReduce of the [64, 768] partials.

Host-side preprocessing (input sharding/packing):
  - h0 = x*emb_w + emb_b precomputed in the on-chip "CP" layout
  - GCN linear merged into conv weights (mw_t = cw_t @ gw), block-diagonal per
    batch-plane; gb bias pre-convolved into gbc (zero-pad edge effects included)
  - lout_w column chunk pre-transposed to [24576, 768] bf16 per core

On-chip layouts (per core; plane = batch half, plane0 = local batches 0-3):
  CP: [128 = (plane, c), (b'=4, n=64, wp=54)]  (w padded by 3 both sides)
  NP: [128 = (plane, n), (b'=4, c=64, w=48)]
Layout flips go through DRAM scratch with strided DMAs (no compute engines).
"""

import os
import numpy as np
import ml_dtypes

import concourse.bass as bass
import concourse.tile as tile
from concourse import mybir, bacc
from concourse.bass_utils import run_bass_kernel_spmd

F32 = mybir.dt.float32
F32R = mybir.dt.float32r
BF16 = mybir.dt.bfloat16
AF = mybir.ActivationFunctionType
ALU = mybir.AluOpType

B, W, N, C, H, QK, HOR = 64, 48, 64, 64, 64, 32, 12
NCORES = 8
BL = B // NCORES          # 8 local batches
BP = BL // 2              # 4 batches per plane
SEQ = BL * N              # 512 sequences per core
WP = W + 6                # padded w
KCH = W * N * C // NCORES # 24576 reduction chunk per core
KS = [3, 5, 7]
TOFF = [0, 3, 8]
CPF = BP * N * WP         # 13824
NPF = BP * C * W          # 12288
RO = N * HOR              # 768


def _ap(base_ap, off, dims):
    """AP with same partition dim as base_ap, extra element offset, given free dims."""
    return bass.AP(tensor=base_ap.tensor, offset=base_ap.offset + off,
                   ap=[list(base_ap.ap[0])] + [list(d) for d in dims])


def _build():
    nc = bacc.Bacc("TRN2", target_bir_lowering=False, debug=False, num_devices=NCORES)
    P = nc.declare_dram_parameter

    x1 = P("x1", [1, W * SEQ], BF16, isOutput=False)
    state0 = P("state0", [64, SEQ], F32R, isOutput=False)
    gru_hh_rz = P("gru_hh_rz", [64, 128], F32R, isOutput=False)
    gru_ih_rz = P("gru_ih_rz", [1, 128], BF16, isOutput=False)
    gru_hh_n = P("gru_hh_n", [64, 64], F32R, isOutput=False)
    gru_ih_n = P("gru_ih_n", [1, 64], BF16, isOutput=False)
    bias_r = P("bias_r", [64, 1], F32, isOutput=False)
    bias_z = P("bias_z", [64, 1], F32, isOutput=False)
    bias_ihn = P("bias_ihn", [64, 1], F32, isOutput=False)
    bias_hhn = P("bias_hhn", [64, 1], F32, isOutput=False)
    wqT = P("wqT", [64, QK], F32R, isOutput=False)
    wkT = P("wkT", [64, QK], F32R, isOutput=False)
    wqb = P("wqb", [QK, 1], F32, isOutput=False)
    wkb = P("wkb", [QK, 1], F32, isOutput=False)
    ones64 = P("ones64", [64, 1], F32, isOutput=False)
    onesrow = P("onesrow", [1, SEQ], BF16, isOutput=False)
    bhhn_row = P("bhhn_row", [1, 64], BF16, isOutput=False)
    ident64f = P("ident64f", [64, 64], F32, isOutput=False)
    zeros128 = P("zeros128", [128, 128], F32R, isOutput=False)
    h0cp = P("h0cp", [128, CPF], F32R, isOutput=False)
    mwbd = P("mwbd", [15, 128, 128], F32R, isOutput=False)
    gbc = P("gbc", [3, 128, C * W], F32, isOutput=False)
    identb = P("identb", [64, 64], BF16, isOutput=False)
    identb2 = P("identb2", [128, 64], BF16, isOutput=False)
    zerosb = P("zerosb", [128, 128], BF16, isOutput=False)
    lwT = P("lwT", [KCH, RO], BF16, isOutput=False)
    out = P("out", [B, RO], F32, isOutput=True)

    with tile.TileContext(nc) as tc:
        with tc.tile_pool(name="persist", bufs=1) as pp, \
             tc.tile_pool(name="finw", bufs=20) as fw, \
             tc.tile_pool(name="dram", bufs=1, space="DRAM") as dp:

            mw_sb = pp.tile([128, 15 * 128], F32R, tag="mw")
            mw_src = bass.AP(tensor=mwbd[:].tensor, offset=mwbd[:].offset,
                             ap=[[128, 128], [128 * 128, 15], [1, 128]])
            nc.sync.dma_start(mw_sb[:].rearrange("p (k m) -> p k m", k=15), mw_src)
            id_sb = pp.tile([64, 64], BF16, tag="ident")
            nc.sync.dma_start(id_sb[:], identb[:])
            id2_sb = pp.tile([128, 64], BF16, tag="ident2")
            nc.sync.dma_start(id2_sb[:], identb2[:])
            aggw = []
            for bpi in range(BP):
                t = pp.tile([128, 128], BF16, tag=f"aggw{bpi}")
                nc.sync.dma_start(t[:], zerosb[:])
                aggw.append(t)
            z128 = pp.tile([128, 128], F32R, tag="z128")
            nc.sync.dma_start(z128[:], zeros128[:])

            scr_np = dp.tile([128, NPF], F32, tag="scr_np")
            scr_cp = dp.tile([128, BP * N * W], F32, tag="scr_cp")
            a2a_in = dp.tile([B, KCH], BF16, tag="a2a_in")
            a2a_out = dp.tile([B, KCH], BF16, tag="a2a_out")
            ar_in = dp.tile([B, RO], F32, tag="ar_in")
            ar_out = dp.tile([B, RO], F32, tag="ar_out")

            # ================= GRU =================
            with tc.tile_pool(name="gru", bufs=1) as gp, \
                 tc.tile_pool(name="gwk", bufs=4) as gwk:

                x1_sb = gp.tile([1, W * SEQ], BF16, tag="x1")
                nc.sync.dma_start(x1_sb[:], x1[:])
                state = gp.tile([64, SEQ], F32R, tag="state")
                nc.sync.dma_start(state[:], state0[:])
                in_alls = []
                for ia in range(8):
                    in_t = gp.tile([64, 6 * SEQ], BF16, tag=f"in_all{ia}", name=f"in_all{ia}")
                    in_alls.append(in_t)

                hh_rz = gp.tile([64, 128], F32R, tag="hh_rz")
                nc.sync.dma_start(hh_rz[:], gru_hh_rz[:])
                ih_rz = gp.tile([1, 128], BF16, tag="ih_rz")
                nc.sync.dma_start(ih_rz[:], gru_ih_rz[:])
                hh_n = gp.tile([64, 64], F32R, tag="hh_n")
                nc.sync.dma_start(hh_n[:], gru_hh_n[:])
                ih_n = gp.tile([1, 64], BF16, tag="ih_n")
                nc.sync.dma_start(ih_n[:], gru_ih_n[:])
                b_r = gp.tile([64, 1], F32, tag="b_r")
                nc.sync.dma_start(b_r[:], bias_r[:])
                b_z = gp.tile([64, 1], F32, tag="b_z")
                nc.sync.dma_start(b_z[:], bias_z[:])
                b_ihn = gp.tile([64, 1], F32, tag="b_ihn")
                nc.sync.dma_start(b_ihn[:], bias_ihn[:])
                b_hhn = gp.tile([64, 1], F32, tag="b_hhn")
                nc.sync.dma_start(b_hhn[:], bias_hhn[:])
                ones_row = gp.tile([1, SEQ], BF16, tag="ones_row")
                nc.sync.dma_start(ones_row[:], onesrow[:])
                bhhn_r = gp.tile([1, 64], BF16, tag="bhhn_r")
                nc.sync.dma_start(bhhn_r[:], bhhn_row[:])

                with tc.tile_pool(name="gps", bufs=1, space="PSUM") as gps:
                    # in_ = w_ih_n * x + b_ihn for all steps (bf16)
                    for t in range(W):
                        pin = gps.tile([64, SEQ], F32, tag="pin")
                        nc.tensor.matmul(pin[:], ih_n[:],
                                         x1_sb[0:1, t * SEQ:(t + 1) * SEQ],
                                         start=True, stop=True)
                        nc.scalar.activation(
                            in_alls[t // 6][:, (t % 6) * SEQ:(t % 6 + 1) * SEQ],
                            pin[:], AF.Identity, bias=b_ihn[:])

                    CH = 256
                    for t in range(W):
                        for ch in range(2):
                            cs = ch * CH
                            prz = gps.tile([128, CH], F32, tag=f"prz{ch}")
                            nc.tensor.matmul(prz[:], hh_rz[:], state[:, cs:cs + CH],
                                             start=True, stop=False)
                            nc.tensor.matmul(prz[:], ih_rz[:],
                                             x1_sb[0:1, t * SEQ + cs: t * SEQ + cs + CH],
                                             start=False, stop=True)
                            pn = gps.tile([64, CH], F32, tag=f"pn{ch}")
                            nc.tensor.matmul(pn[:], hh_n[:], state[:, cs:cs + CH],
                                             start=True, stop=False)
                            nc.tensor.matmul(pn[:], bhhn_r[:], ones_row[0:1, 0:CH],
                                             start=False, stop=True)
                            rt = gwk.tile([64, CH], F32, tag=f"rt{ch}")
                            nc.scalar.activation(rt[:], prz[0:64, :], AF.Sigmoid, bias=b_r[:])
                            zt = gwk.tile([64, CH], F32, tag=f"zt{ch}")
                            nc.scalar.activation(zt[:], prz[64:128, :], AF.Sigmoid, bias=b_z[:])
                            t1 = gwk.tile([64, CH], F32, tag=f"t1{ch}")
                            nc.vector.tensor_mul(t1[:], rt[:], pn[:])
                            npre = gwk.tile([64, CH], F32, tag=f"npre{ch}")
                            nc.vector.tensor_add(
                                npre[:], t1[:],
                                in_alls[t // 6][:, (t % 6) * SEQ + cs: (t % 6) * SEQ + cs + CH])
                            nt = gwk.tile([64, CH], F32, tag=f"nt{ch}")
                            nc.scalar.activation(nt[:], npre[:], AF.Tanh)
                            dt_ = gwk.tile([64, CH], F32, tag=f"dt{ch}")
                            nc.vector.tensor_sub(dt_[:], state[0:64, cs:cs + CH].bitcast(F32), nt[:])
                            zd = gwk.tile([64, CH], F32, tag=f"zd{ch}")
                            nc.vector.tensor_mul(zd[:], zt[:], dt_[:])
                            nc.vector.tensor_add(state[0:64, cs:cs + CH], nt[:], zd[:])

                # ---- attention -> Anorm -> aggw quadrants ----
                wq_sb = gp.tile([64, QK], F32R, tag="wq")
                nc.sync.dma_start(wq_sb[:], wqT[:])
                wk_sb = gp.tile([64, QK], F32R, tag="wk")
                nc.sync.dma_start(wk_sb[:], wkT[:])
                wqb_sb = gp.tile([QK, 1], F32, tag="wqb")
                nc.sync.dma_start(wqb_sb[:], wqb[:])
                wkb_sb = gp.tile([QK, 1], F32, tag="wkb")
                nc.sync.dma_start(wkb_sb[:], wkb[:])
                ones_sb = gp.tile([64, 1], F32, tag="ones")
                nc.sync.dma_start(ones_sb[:], ones64[:])
                idf_sb = gp.tile([64, 64], F32, tag="idf")
                nc.sync.dma_start(idf_sb[:], ident64f[:])

                with tc.tile_pool(name="aps", bufs=1, space="PSUM") as aps:
                    pq = aps.tile([QK, SEQ], F32, tag="pq")
                    nc.tensor.matmul(pq[:], wq_sb[:], state[:], start=True, stop=True)
                    qt = gp.tile([QK, SEQ], F32, tag="qt")
                    nc.scalar.activation(qt[:], pq[:], AF.Identity, bias=wqb_sb[:])
                    pk = aps.tile([QK, SEQ], F32, tag="pk")
                    nc.tensor.matmul(pk[:], wk_sb[:], state[:], start=True, stop=True)
                    kt = gp.tile([QK, SEQ], F32, tag="kt")
                    nc.scalar.activation(kt[:], pk[:], AF.Identity, bias=wkb_sb[:])

                    for b in range(BL):
                        ps_ = aps.tile([64, 64], F32, tag="ps_s")
                        nc.tensor.matmul(ps_[:], qt[:, b * 64:(b + 1) * 64],
                                         kt[:, b * 64:(b + 1) * 64], start=True, stop=True)
                        s_sb = gwk.tile([64, 64], F32, tag="s_sb")
                        nc.scalar.activation(s_sb[:], ps_[:], AF.Identity,
                                             scale=1.0 / float(np.sqrt(QK)))
                        mx = gwk.tile([64, 1], F32, tag="mx")
                        nc.vector.tensor_reduce(out=mx[:], in_=s_sb[:], op=ALU.max,
                                                axis=mybir.AxisListType.X)
                        nmx = gwk.tile([64, 1], F32, tag="nmx")
                        nc.vector.tensor_scalar_mul(nmx[:], mx[:], -1.0)
                        ex = gwk.tile([64, 64], F32, tag="ex")
                        nc.scalar.activation(ex[:], s_sb[:], AF.Exp, bias=nmx[:])
                        sm = gwk.tile([64, 1], F32, tag="sm")
                        nc.vector.tensor_reduce(out=sm[:], in_=ex[:], op=ALU.add,
                                                axis=mybir.AxisListType.X)
                        rs = gwk.tile([64, 1], F32, tag="rs")
                        nc.vector.reciprocal(rs[:], sm[:])
                        a_sb = gwk.tile([64, 64], F32, tag="a_sb")
                        nc.vector.tensor_scalar_mul(a_sb[:], ex[:], rs[:])
                        pc = aps.tile([64, 1], F32, tag="pc")
                        nc.tensor.matmul(pc[:], a_sb[:], ones_sb[:], start=True, stop=True)
                        dsq = gwk.tile([64, 1], F32, tag="dsq")
                        nc.scalar.activation(dsq[:], pc[:], AF.Sqrt)
                        dinv = gwk.tile([64, 1], F32, tag="dinv")
                        nc.vector.reciprocal(dinv[:], dsq[:])
                        pr = aps.tile([1, 64], F32, tag="pr")
                        nc.tensor.matmul(pr[:], dinv[:], idf_sb[:], start=True, stop=True)
                        dinvr = gwk.tile([1, 64], F32, tag="dinvr")
                        nc.scalar.activation(dinvr[:], pr[:], AF.Identity)
                        po = aps.tile([64, 64], F32, tag="po")
                        nc.tensor.matmul(po[:], dinvr[:], dinvr[:], start=True, stop=True)
                        quad = aggw[b % BP][0:64, 0:64] if b < BP else aggw[b - BP][64:128, 64:128]
                        nc.vector.tensor_mul(quad, a_sb[:], po[:])

            # ================= blocks =================
            with tc.tile_pool(name="blk", bufs=1) as bkp, \
                 tc.tile_pool(name="stg", bufs=3) as stg, \
                 tc.tile_pool(name="gbp", bufs=1) as gbp, \
                 tc.tile_pool(name="bwk", bufs=5) as bw, \
                 tc.tile_pool(name="bps", bufs=3, space="PSUM") as bps, \
                 tc.tile_pool(name="bps2", bufs=2, space="PSUM") as bps2, tc.tile_pool(name="bps3", bufs=3, space="PSUM") as bps3:

                hcp = bkp.tile([128, CPF], F32R, tag="hcp")
                nc.sync.dma_start(hcp[:], h0cp[:])

                for blk in range(3):
                    k = KS[blk]
                    pad = k // 2
                    gbc_sb = gbp.tile([128, C * W], F32, tag="gbc")
                    nc.sync.dma_start(gbc_sb[:], gbc[blk])
                    last = blk == 2

                    # conv (merged gcn linear): CP -> psum -> s1(bf16) -> PE-transpose -> AGG
                    for bl in range(BP):
                        if last:
                            hst = stg.tile([128, C * W], BF16, tag="hst")
                        else:
                            hst = stg.tile([128, C * W], F32, tag="hst")
                        for wg in range(8):
                            base = bl * N * WP + 3 + wg * 6
                            p1 = bps.tile([128, N, 6], F32, tag="p1")
                            for t in range(k):
                                rhs = _ap(hcp[:], base + (t - pad), [[WP, N], [1, 6]])
                                nc.tensor.matmul(
                                    p1[:],
                                    mw_sb[:, (TOFF[blk] + t) * 128:(TOFF[blk] + t + 1) * 128],
                                    rhs, start=(t == 0), stop=(t == k - 1))
                            s1 = bw.tile([128, N, 6], BF16, tag="s1")
                            nc.scalar.activation(s1[:], p1[:], AF.Identity)
                            p1t = bps2.tile([128, 6, C], BF16, tag="p1t")
                            for pl in range(2):
                                for wl in range(6):
                                    sl = s1[pl * 64:pl * 64 + 64, :, wl]
                                    tin = bass.AP(tensor=sl.tensor, offset=sl.offset,
                                                  ap=[list(sl.ap[0]), [6, N]])
                                    nc.tensor.transpose(p1t[pl * 64:pl * 64 + 64, wl, :],
                                                        tin, id2_sb[pl * 64:pl * 64 + 64, :])
                            hnt = bw.tile([128, 6, C], BF16, tag="hnt")
                            nc.scalar.activation(hnt[:], p1t[:], AF.Identity)
                            p2 = bps3.tile([128, 6 * C], F32, tag="p2")
                            nc.tensor.matmul(p2[:], aggw[bl][:],
                                             hnt[:].rearrange("p a b -> p (a b)"),
                                             start=True, stop=True)
                            s2 = bw.tile([128, 6 * C], F32, tag="s2")
                            nc.vector.tensor_add(s2[:], p2[:],
                                                 gbc_sb[:, wg * 384:(wg + 1) * 384])
                            if last:
                                nc.scalar.activation(hst[:, wg * 384:(wg + 1) * 384],
                                                     s2[:], AF.Lrelu)
                            else:
                                ldst = _ap(hst[:], wg * 6, [[1, 6], [W, C]])
                                sin = s2[:].rearrange("p (a b) -> p a b", a=6)
                                nc.scalar.activation(ldst, sin, AF.Lrelu)
                        for pl in range(2):
                            sl = hst[pl * 64:pl * 64 + 64, :]
                            if last:
                                for jj in range(8):
                                    asrc = bass.AP(tensor=sl.tensor,
                                                   offset=sl.offset + jj * 6 * C,
                                                   ap=[list(sl.ap[0])] + [[C, 6], [1, C]])
                                    adst = bass.AP(
                                        tensor=a2a_in[:].tensor,
                                        offset=a2a_in[:].offset + (jj * BL + pl * BP + bl) * KCH,
                                        ap=[[C, N], [N * C, 6], [1, C]])
                                    nc.sync.dma_start(adst, asrc)
                            else:
                                fsrc = bass.AP(tensor=sl.tensor, offset=sl.offset,
                                               ap=[list(sl.ap[0])] + [[W, C], [1, W]])
                                fdst = bass.AP(
                                    tensor=scr_cp[:].tensor,
                                    offset=scr_cp[:].offset + pl * 64 * (BP * N * W) + bl * N * W,
                                    ap=[[W, 64], [BP * N * W, 64], [1, W]])
                                nc.sync.dma_start(fdst, fsrc)

                    if not last:
                        hcp = bkp.tile([128, CPF], F32R, tag="hcp")
                        for off in (0, 3 + W):
                            zdst = _ap(hcp[:], off, [[WP, BP * N], [1, 3]])
                            zsrc = _ap(z128[:], 0, [[0, BP * N], [1, 3]])
                            nc.vector.tensor_copy(zdst, zsrc.bitcast(F32))
                        idst = _ap(hcp[:], 3, [[WP, BP * N], [1, W]])
                        isrc = bass.AP(tensor=scr_cp[:].tensor, offset=scr_cp[:].offset,
                                       ap=[[BP * N * W, 128], [W, BP * N], [1, W]])
                        nc.sync.dma_start(idst, isrc.bitcast(F32R))

            # ================= A2A + final linear + AR =================
            nc.gpsimd.collective_compute(
                "AllToAll", ALU.bypass,
                replica_groups=[list(range(NCORES))],
                ins=[a2a_in.opt()], outs=[a2a_out.opt()])

            with tc.tile_pool(name="fin", bufs=8) as fp, \
                 tc.tile_pool(name="fps", bufs=2, space="PSUM") as fps, \
                 tc.tile_pool(name="fpo", bufs=1, space="PSUM") as fpo:

                pout = []
                for h in range(2):
                    po_t = fpo.tile([64, 384], F32, tag=f"pout{h}", name=f"pout{h}")
                    pout.append(po_t)
                NKT = KCH // 128
                for kt_ in range(NKT):
                    wt = fw.tile([128, RO], BF16, tag="wt")
                    weng = nc.sync if kt_ % 2 == 0 else nc.gpsimd
                    weng.dma_start(wt[:], lwT[kt_ * 128:(kt_ + 1) * 128, :])
                    ho = fp.tile([64, 128], BF16, tag="ho")
                    nc.scalar.dma_start(ho[:], a2a_out[:, kt_ * 128:(kt_ + 1) * 128])
                    pt = fps.tile([128, 64], BF16, tag="pt")
                    nc.tensor.transpose(pt[:], ho[:], id_sb[:])
                    ht = fp.tile([128, 64], BF16, tag="ht")
                    nc.scalar.activation(ht[:], pt[:], AF.Identity)
                    for hh in range(2):
                        nc.tensor.matmul(pout[hh][:], ht[:], wt[:, hh * 384:(hh + 1) * 384],
                                         start=(kt_ == 0), stop=(kt_ == NKT - 1),
                                         skip_group_check=True)
                oo = fp.tile([64, RO], F32, tag="oo")
                for hh in range(2):
                    nc.scalar.activation(oo[:, hh * 384:(hh + 1) * 384], pout[hh][:], AF.Identity)
                nc.sync.dma_start(ar_in[:], oo[:])
                nc.gpsimd.collective_compute(
                    "AllReduce", ALU.add,
                    replica_groups=[list(range(NCORES))],
                    ins=[ar_in.opt()], outs=[ar_out.opt()])
                oo2 = fp.tile([64, RO], F32, tag="oo2")
                nc.sync.dma_start(oo2[:], ar_out[:])
                nc.sync.dma_start(out[:], oo2[:])

    nc.compile()
    return nc


_NC = None


def _host_prep(inputs):
    f32 = np.float32
    bf16 = ml_dtypes.bfloat16
    x = np.asarray(inputs["x"], f32)
    w_ih = np.asarray(inputs["gru_w_ih"], f32)
    w_hh = np.asarray(inputs["gru_w_hh"], f32)
    b_ih = np.asarray(inputs["gru_b_ih"], f32)
    b_hh = np.asarray(inputs["gru_b_hh"], f32)

    shared = {
        "state0": np.zeros((64, SEQ), f32),
        "gru_hh_rz": np.ascontiguousarray(w_hh[0:128].T),
        "gru_ih_rz": np.ascontiguousarray(w_ih[0:128, 0][None, :]).astype(bf16),
        "gru_hh_n": np.ascontiguousarray(w_hh[128:192].T),
        "gru_ih_n": np.ascontiguousarray(w_ih[128:192, 0][None, :]).astype(bf16),
        "bias_r": (b_ih + b_hh)[0:64, None].copy(),
        "bias_z": (b_ih + b_hh)[64:128, None].copy(),
        "bias_ihn": b_ih[128:192, None].copy(),
        "bias_hhn": b_hh[128:192, None].copy(),
        "wqT": np.ascontiguousarray(np.asarray(inputs["wq_w"], f32).T),
        "wkT": np.ascontiguousarray(np.asarray(inputs["wk_w"], f32).T),
        "wqb": np.asarray(inputs["wq_b"], f32)[:, None].copy(),
        "wkb": np.asarray(inputs["wk_b"], f32)[:, None].copy(),
        "ones64": np.ones((64, 1), f32),
        "onesrow": np.ones((1, SEQ)).astype(bf16),
        "bhhn_row": b_hh[128:192][None, :].astype(bf16),
        "ident64f": np.eye(64, dtype=f32),
        "zeros128": np.zeros((128, 128), f32),
        "identb": np.eye(64).astype(bf16),
        "identb2": np.vstack([np.eye(64), np.eye(64)]).astype(bf16),
        "zerosb": np.zeros((128, 128)).astype(bf16),
    }

    mwbd = np.zeros((15, 128, 128), f32)
    gbc_a = np.zeros((3, 128, C * W), f32)
    for i in range(3):
        gw_ = np.asarray(inputs[f"gcn_w{i}"], f32)
        gb = np.asarray(inputs[f"gcn_b{i}"], f32)
        cw = np.asarray(inputs[f"conv_w{i}"], f32)
        cb = np.asarray(inputs[f"conv_b{i}"], f32)
        k = KS[i]
        pad = k // 2
        for t in range(k):
            q = (cw[:, :, t] @ gw_).T         # lhsT quadrant [c_in, c_out]
            mwbd[TOFF[i] + t, 0:64, 0:64] = q
            mwbd[TOFF[i] + t, 64:128, 64:128] = q
        cgt = np.einsum("oit,i->ot", cw, gb)  # [o, k]
        g_ = np.zeros((C, W), f32)
        for w in range(W):
            for t in range(k):
                if 0 <= w + t - pad < W:
                    g_[:, w] += cgt[:, t]
        g_ += cb[:, None]
        gbc_a[i] = np.tile(g_.T.reshape(W * C), (128, 1))

    shared["mwbd"] = mwbd
    shared["gbc"] = gbc_a

    emb_w = np.asarray(inputs["emb_w"], f32)
    emb_b = np.asarray(inputs["emb_b"], f32)
    lout_w = np.asarray(inputs["lout_w"], f32)

    in_maps = []
    for c_ in range(NCORES):
        xc = x[c_ * BL:(c_ + 1) * BL]
        m = dict(shared)
        m["x1"] = np.ascontiguousarray(
            xc.transpose(1, 0, 2).reshape(1, W * SEQ)).astype(bf16)
        h0 = xc[..., None] * emb_w + emb_b                  # [8, 48, 64, 64]
        hcp_h = np.zeros((2, 64, BP, N, WP), f32)
        hsrc = h0.reshape(2, BP, W, N, C).transpose(0, 4, 1, 3, 2)
        hcp_h[:, :, :, :, 3:3 + W] = hsrc
        m["h0cp"] = np.ascontiguousarray(hcp_h.reshape(128, CPF))
        lw = lout_w[:, c_ * KCH:(c_ + 1) * KCH]
        m["lwT"] = np.ascontiguousarray(lw.T).astype(bf16)
        in_maps.append(m)
    return in_maps


def kernel_with_stats(**inputs):
    global _NC
    if _NC is None:
        _NC = _build()
    in_maps = _host_prep(inputs)
    trace = os.environ.get("KERNEL_TRACE", "") == "1"
    res = run_bass_kernel_spmd(_NC, in_maps, core_ids=list(range(NCORES)), trace=trace)
    out = res.results[0]["out"] + np.asarray(inputs["lout_b"], np.float32)[None, :]
    return out.reshape(B, HOR, N).astype(np.float32), res


def kernel(**inputs):
    o, _ = kernel_with_stats(**inputs)
    return o



# revision 12
# speedup vs baseline: 1.2322x; 1.2322x over previous
"""Trainium2 Bass kernel for BaselineBlockNetSingleGraph (GRU + attention-GCN + convs + big linear).

v2 optimizations over baseline:
  - GRU: combined sigmoid over [r;z], fused scalar_tensor_tensor blend,
    z*h on gpsimd, in_all precompute interleaved into the step loop.
  - Attention batched per batch-plane-pair (4 iters instead of 8).
  - Blocks: bias folded into PE accumulate matmuls (rank-1 rhs rows) or DVE
    broadcast adds; block1 runs aggregation-first so only ONE DRAM layout flip
    is needed for the whole net; bf16 activations everywhere.
  - Final linear: lout_w streamed as 48 x 768KB packed DMAs with several
    prefetched from kernel start; a2a_out preloaded to SBUF once.

Layouts per core (plane = batch half, b' = batch-in-plane 0..3):
  CP: [128 = (plane, c), (b'=4, n=64, wp=54)]  (w padded by 3 both sides)
  NP: [128 = (plane, n), (b'=4, c=64, w=48)]
"""

import os
import numpy as np
import ml_dtypes

import concourse.bass as bass
import concourse.tile as tile
from concourse import mybir, bacc
from concourse.bass_utils import run_bass_kernel_spmd

F32 = mybir.dt.float32
F32R = mybir.dt.float32r
BF16 = mybir.dt.bfloat16
AF = mybir.ActivationFunctionType
ALU = mybir.AluOpType

B, W, N, C, H, QK, HOR = 64, 48, 64, 64, 64, 32, 12
NCORES = 8
BL = B // NCORES          # 8 local batches
BP = BL // 2              # 4 batches per plane
SEQ = BL * N              # 512 sequences per core
WP = W + 6                # padded w
KCH = W * N * C // NCORES # 24576 reduction chunk per core
KS = [3, 5, 7]
TOFF = [0, 3, 8]
CPF = BP * N * WP         # 13824
RO = N * HOR              # 768
CH = 256                  # GRU chunk width
NWT = 48                  # weight mega-tiles (4 k-tiles each)
WPRE = 7                  # prefetched weight tiles


def _ap(base_ap, off, dims):
    return bass.AP(tensor=base_ap.tensor, offset=base_ap.offset + off,
                   ap=[list(base_ap.ap[0])] + [list(d) for d in dims])


def _build():
    nc = bacc.Bacc("TRN2", target_bir_lowering=False, debug=False, num_devices=NCORES)
    P = nc.declare_dram_parameter

    x1 = P("x1", [1, W * SEQ], BF16, isOutput=False)
    state0 = P("state0", [64, SEQ], F32R, isOutput=False)
    gru_hh_rz = P("gru_hh_rz", [64, 128], F32R, isOutput=False)
    gru_ih_rz = P("gru_ih_rz", [1, 128], BF16, isOutput=False)
    gru_hh_n = P("gru_hh_n", [64, 64], F32R, isOutput=False)
    gru_ih_n = P("gru_ih_n", [1, 64], BF16, isOutput=False)
    bias_r = P("bias_r", [64, 1], F32, isOutput=False)
    bias_z = P("bias_z", [64, 1], F32, isOutput=False)
    bias_ihn = P("bias_ihn", [64, 1], F32, isOutput=False)
    bhhn_row = P("bhhn_row", [1, 64], BF16, isOutput=False)
    onesrow = P("onesrow", [1, SEQ], BF16, isOutput=False)
    wqT = P("wqT", [64, QK], F32R, isOutput=False)
    wkT = P("wkT", [64, QK], F32R, isOutput=False)
    wqb = P("wqb", [QK, 1], F32, isOutput=False)
    wkb = P("wkb", [QK, 1], F32, isOutput=False)
    ones128 = P("ones128", [128, 1], F32, isOutput=False)
    idf128 = P("idf128", [128, 64], F32, isOutput=False)
    onescol_b = P("onescol_b", [1, 128], BF16, isOutput=False)
    identb = P("identb", [64, 64], BF16, isOutput=False)
    identb2 = P("identb2", [128, 64], BF16, isOutput=False)
    zerosb = P("zerosb", [128, 128], BF16, isOutput=False)
    h0cp = P("h0cp", [128, CPF], BF16, isOutput=False)
    mwbd = P("mwbd", [15, 128, 128], BF16, isOutput=False)
    b0row = P("b0row", [1, C * W], BF16, isOutput=False)
    b2row = P("b2row", [1, C * W], BF16, isOutput=False)
    gbc1 = P("gbc1", [128, W], F32, isOutput=False)
    lwT = P("lwT", [KCH, RO], BF16, isOutput=False)
    out = P("out", [B, RO], F32, isOutput=True)

    with tile.TileContext(nc) as tc:
        with tc.tile_pool(name="persist", bufs=1) as pp, \
             tc.tile_pool(name="wpool", bufs=WPRE) as wp, \
             tc.tile_pool(name="bk", bufs=1) as bk, \
             tc.tile_pool(name="dram", bufs=1, space="DRAM") as dp:

            # ---------- persistent small tensors ----------
            mw_sb = pp.tile([128, 15 * 128], BF16, tag="mw")
            mw_src = bass.AP(tensor=mwbd[:].tensor, offset=mwbd[:].offset,
                             ap=[[128, 128], [128 * 128, 15], [1, 128]])
            nc.sync.dma_start(mw_sb[:].rearrange("p (k m) -> p k m", k=15), mw_src)
            id_sb = pp.tile([64, 64], BF16, tag="ident")
            nc.sync.dma_start(id_sb[:], identb[:])
            id2_sb = pp.tile([128, 64], BF16, tag="ident2")
            nc.sync.dma_start(id2_sb[:], identb2[:])
            zb_sb = pp.tile([128, 128], BF16, tag="zb")
            nc.sync.dma_start(zb_sb[:], zerosb[:])
            onescol = pp.tile([1, 128], BF16, tag="onescol")
            nc.sync.dma_start(onescol[:], onescol_b[:])
            b0_sb = pp.tile([1, C * W], BF16, tag="b0row")
            nc.sync.dma_start(b0_sb[:], b0row[:])
            b2_sb = pp.tile([1, C * W], BF16, tag="b2row")
            nc.sync.dma_start(b2_sb[:], b2row[:])
            gbc1_sb = pp.tile([128, W], F32, tag="gbc1")
            nc.sync.dma_start(gbc1_sb[:], gbc1[:])
            aggw = []
            for bpi in range(BP):
                t = pp.tile([128, 128], BF16, tag=f"aggw{bpi}")
                nc.sync.dma_start(t[:], zerosb[:])
                aggw.append(t)
            state = pp.tile([64, SEQ], F32R, tag="state")
            nc.sync.dma_start(state[:], state0[:])

            # big stage tensors; hcpA is reused for hcp0 then hcp2
            hcp0 = bk.tile([128, CPF], BF16, tag="hcpA")

            scr = dp.tile([128, BP * N * W], BF16, tag="scr")
            a2a_in = dp.tile([B, KCH], BF16, tag="a2a_in")
            a2a_out = dp.tile([B, KCH], BF16, tag="a2a_out")
            ar_in = dp.tile([B, RO], F32, tag="ar_in")
            ar_out = dp.tile([B, RO], F32, tag="ar_out")

            wt_tiles = {}

            def _issue_wt(j, eng):
                wt_tiles[j] = wp.tile([128, 4 * RO], BF16, tag="wt4", name=f"wt4_{j}")
                src = bass.AP(tensor=lwT[:].tensor,
                              offset=lwT[:].offset + j * 512 * RO,
                              ap=[[RO, 128], [128 * RO, 4], [1, RO]])
                eng.dma_start(wt_tiles[j][:].rearrange("p (k m) -> p k m", k=4), src)

            # ================= GRU =================
            with tc.tile_pool(name="gru", bufs=1) as gp, \
                 tc.tile_pool(name="inp", bufs=8) as inp, \
                 tc.tile_pool(name="gwk", bufs=1) as gwk, \
                 tc.tile_pool(name="gps", bufs=1, space="PSUM") as gps, \
                 tc.tile_pool(name="gps2", bufs=2, space="PSUM") as gps2:

                x1_sb = gp.tile([1, W * SEQ], BF16, tag="x1")
                nc.sync.dma_start(x1_sb[:], x1[:])
                hh_rz = gp.tile([64, 128], F32R, tag="hh_rz")
                nc.sync.dma_start(hh_rz[:], gru_hh_rz[:])
                ih_rz = gp.tile([1, 128], BF16, tag="ih_rz")
                nc.sync.dma_start(ih_rz[:], gru_ih_rz[:])
                hh_n = gp.tile([64, 64], F32R, tag="hh_n")
                nc.sync.dma_start(hh_n[:], gru_hh_n[:])
                ih_n = gp.tile([1, 64], BF16, tag="ih_n")
                nc.sync.dma_start(ih_n[:], gru_ih_n[:])
                b_r = gp.tile([64, 1], F32, tag="b_r")
                nc.sync.dma_start(b_r[:], bias_r[:])
                b_z = gp.tile([64, 1], F32, tag="b_z")
                nc.sync.dma_start(b_z[:], bias_z[:])
                b_ihn = gp.tile([64, 1], F32, tag="b_ihn")
                nc.sync.dma_start(b_ihn[:], bias_ihn[:])
                bhhn_r = gp.tile([1, 64], BF16, tag="bhhn_r")
                nc.sync.dma_start(bhhn_r[:], bhhn_row[:])
                ones_row = gp.tile([1, SEQ], BF16, tag="ones_row")
                nc.sync.dma_start(ones_row[:], onesrow[:])

                # weight prefetch + h0 load (behind the small GRU loads)
                for j in range(WPRE):
                    _issue_wt(j, nc.sync)
                nc.sync.dma_start(hcp0[:], h0cp[:])

                in_tiles = [None] * W

                def _pre_in(t):
                    pin = gps2.tile([64, SEQ], F32, tag="pin")
                    nc.tensor.matmul(pin[:], ih_n[:],
                                     x1_sb[0:1, t * SEQ:(t + 1) * SEQ],
                                     start=True, stop=True)
                    in_tiles[t] = inp.tile([64, SEQ], BF16, tag="inall",
                                           name=f"inall_{t}")
                    nc.scalar.activation(in_tiles[t][:], pin[:], AF.Identity,
                                         bias=b_ihn[:])

                LOOK = 6
                for t in range(LOOK):
                    _pre_in(t)

                for t in range(W):
                    for ch in range(2):
                        cs = ch * CH
                        ssl = state[:, cs:cs + CH]
                        prz = gps.tile([128, CH], F32, tag=f"prz{ch}")
                        nc.tensor.matmul(prz[:], hh_rz[:], ssl,
                                         start=True, stop=False)
                        nc.tensor.matmul(prz[:], ih_rz[:],
                                         x1_sb[0:1, t * SEQ + cs: t * SEQ + cs + CH],
                                         start=False, stop=True)
                        pn = gps.tile([64, CH], F32, tag=f"pn{ch}")
                        nc.tensor.matmul(pn[:], hh_n[:], ssl,
                                         start=True, stop=False)
                        nc.tensor.matmul(pn[:], bhhn_r[:], ones_row[0:1, 0:CH],
                                         start=False, stop=True)
                        rr = gwk.tile([64, CH], F32, tag=f"rr{ch}")
                        nc.scalar.activation(rr[:], prz[0:64, :], AF.Sigmoid, bias=b_r[:])
                        zz = gwk.tile([64, CH], F32, tag=f"zz{ch}")
                        nc.scalar.activation(zz[:], prz[64:128, :], AF.Sigmoid, bias=b_z[:])
                        # u = z * h_old (gpsimd, off critical path)
                        u = gwk.tile([64, CH], F32, tag=f"u{ch}")
                        nc.gpsimd.tensor_mul(u[:], zz[:], ssl.bitcast(F32))
                        t1 = gwk.tile([64, CH], F32, tag=f"t1{ch}")
                        nc.vector.tensor_mul(t1[:], rr[:], pn[:])
                        npre = gwk.tile([64, CH], F32, tag=f"npre{ch}")
                        nc.vector.tensor_add(npre[:], t1[:],
                                             in_tiles[t][:, cs:cs + CH])
                        nt = gwk.tile([64, CH], F32, tag=f"nt{ch}")
                        nc.scalar.activation(nt[:], npre[:], AF.Tanh)
                        # m = (z - 1) * nt ; h' = u - m = z*h + (1-z)*nt
                        m = gwk.tile([64, CH], F32, tag=f"m{ch}")
                        nc.vector.scalar_tensor_tensor(
                            m[:], zz[:], 1.0, nt[:],
                            op0=ALU.subtract, op1=ALU.mult)
                        nc.vector.tensor_sub(ssl, u[:], m[:])
                    if t + LOOK < W:
                        _pre_in(t + LOOK)

            # ================= attention + block0 (conv+agg) =================
            with tc.tile_pool(name="bknp", bufs=1) as bknp:
              hnp1 = bknp.tile([128, BP * C * W], BF16, tag="hnp1")
              with tc.tile_pool(name="att", bufs=1) as at, \
                 tc.tile_pool(name="awk", bufs=2) as awk, \
                 tc.tile_pool(name="hnt", bufs=4) as hntp, \
                 tc.tile_pool(name="aps", bufs=1, space="PSUM") as aps, \
                 tc.tile_pool(name="bps", bufs=1, space="PSUM") as bps, \
                 tc.tile_pool(name="bps2", bufs=1, space="PSUM") as bps2, \
                 tc.tile_pool(name="bps3", bufs=1, space="PSUM") as bps3:

                wq_sb = at.tile([64, QK], F32R, tag="wq")
                nc.sync.dma_start(wq_sb[:], wqT[:])
                wk_sb = at.tile([64, QK], F32R, tag="wk")
                nc.sync.dma_start(wk_sb[:], wkT[:])
                wqb_sb = at.tile([QK, 1], F32, tag="wqb")
                nc.sync.dma_start(wqb_sb[:], wqb[:])
                wkb_sb = at.tile([QK, 1], F32, tag="wkb")
                nc.sync.dma_start(wkb_sb[:], wkb[:])
                ones_sb = at.tile([128, 1], F32, tag="ones")
                nc.sync.dma_start(ones_sb[:], ones128[:])
                idf_sb = at.tile([128, 64], F32, tag="idf")
                nc.sync.dma_start(idf_sb[:], idf128[:])

                pq = aps.tile([QK, SEQ], F32, tag="pqk")
                nc.tensor.matmul(pq[:], wq_sb[:], state[:], start=True, stop=True)
                qt = at.tile([QK, SEQ], F32, tag="qt")
                nc.scalar.activation(qt[:], pq[:], AF.Identity, bias=wqb_sb[:])
                pk = aps.tile([QK, SEQ], F32, tag="pqk")
                nc.tensor.matmul(pk[:], wk_sb[:], state[:], start=True, stop=True)
                kt = at.tile([QK, SEQ], F32, tag="kt")
                nc.scalar.activation(kt[:], pk[:], AF.Identity, bias=wkb_sb[:])

                def _attn(bl):
                    o0, o1 = bl * 64, (BP + bl) * 64
                    ps2 = aps.tile([128, 64], F32, tag="ps2")
                    nc.tensor.matmul(ps2[0:64, :], qt[:, o0:o0 + 64],
                                     kt[:, o0:o0 + 64], start=True, stop=True)
                    nc.tensor.matmul(ps2[64:128, :], qt[:, o1:o1 + 64],
                                     kt[:, o1:o1 + 64], start=True, stop=True)
                    s2 = awk.tile([128, 64], F32, tag="s2a")
                    nc.scalar.activation(s2[:], ps2[:], AF.Identity,
                                         scale=1.0 / float(np.sqrt(QK)))
                    mx = awk.tile([128, 1], F32, tag="mx")
                    nc.vector.tensor_reduce(out=mx[:], in_=s2[:], op=ALU.max,
                                            axis=mybir.AxisListType.X)
                    nmx = awk.tile([128, 1], F32, tag="nmx")
                    nc.vector.tensor_scalar_mul(nmx[:], mx[:], -1.0)
                    ex = awk.tile([128, 64], F32, tag="ex")
                    sm = awk.tile([128, 1], F32, tag="sm")
                    nc.scalar.activation(ex[:], s2[:], AF.Exp, bias=nmx[:],
                                         accum_out=sm[:])
                    rs = awk.tile([128, 1], F32, tag="rs")
                    nc.vector.reciprocal(rs[:], sm[:])
                    a2 = awk.tile([128, 64], F32, tag="a2")
                    nc.vector.tensor_scalar_mul(a2[:], ex[:], rs[:])
                    pc2 = aps.tile([128, 1], F32, tag="pc2")
                    nc.tensor.matmul(pc2[0:64, :], a2[0:64, :], ones_sb[0:64, :],
                                     start=True, stop=True)
                    nc.tensor.matmul(pc2[64:128, :], a2[64:128, :],
                                     ones_sb[64:128, :], start=True, stop=True)
                    dsq = awk.tile([128, 1], F32, tag="dsq")
                    nc.scalar.activation(dsq[:], pc2[:], AF.Sqrt)
                    dinv = awk.tile([128, 1], F32, tag="dinv")
                    nc.vector.reciprocal(dinv[:], dsq[:])
                    po2 = aps.tile([128, 64], F32, tag="po2")
                    for pl in range(2):
                        pr = aps.tile([1, 64], F32, tag="pr")
                        nc.tensor.matmul(pr[:], dinv[pl * 64:(pl + 1) * 64, :],
                                         idf_sb[pl * 64:(pl + 1) * 64, :],
                                         start=True, stop=True)
                        dinvr = awk.tile([1, 64], F32, tag=f"dinvr{pl}")
                        nc.scalar.activation(dinvr[:], pr[:], AF.Identity)
                        nc.tensor.matmul(po2[pl * 64:(pl + 1) * 64, :],
                                         dinvr[:], dinvr[:], start=True, stop=True)
                    nc.vector.tensor_mul(aggw[bl][0:64, 0:64],
                                         a2[0:64, :], po2[0:64, :])
                    nc.vector.tensor_mul(aggw[bl][64:128, 64:128],
                                         a2[64:128, :], po2[64:128, :])

                def _conv0(bl, wg):
                    base = bl * N * WP + 3 + wg * 6
                    p1 = bps.tile([128, N, 6], F32, tag="p1")
                    for tt in range(3):
                        rhs = _ap(hcp0[:], base + (tt - 1), [[WP, N], [1, 6]])
                        nc.tensor.matmul(
                            p1[:], mw_sb[:, (TOFF[0] + tt) * 128:(TOFF[0] + tt + 1) * 128],
                            rhs, start=(tt == 0), stop=(tt == 2))
                    s1 = awk.tile([128, N, 6], BF16, tag="s1z")
                    nc.vector.tensor_copy(s1[:], p1[:])
                    p1t = bps2.tile([128, 6, C], BF16, tag="p1t")
                    for pl in range(2):
                        for wl in range(6):
                            sl = s1[pl * 64:pl * 64 + 64, :, wl]
                            tin = bass.AP(tensor=sl.tensor, offset=sl.offset,
                                          ap=[list(sl.ap[0]), [6, N]])
                            nc.tensor.transpose(p1t[pl * 64:pl * 64 + 64, wl, :],
                                                tin, id2_sb[pl * 64:pl * 64 + 64, :])
                    ht_ = hntp.tile([128, 6, C], BF16, tag="hnt0")
                    nc.scalar.activation(ht_[:], p1t[:], AF.Identity)
                    return ht_

                def _agg0(bl, wg, ht_):
                    p2 = bps3.tile([128, 6 * C], F32, tag="p2")
                    nc.tensor.matmul(p2[:], aggw[bl][:],
                                     ht_[:].rearrange("p a b -> p (a b)"),
                                     start=True, stop=False)
                    nc.tensor.matmul(p2[:], onescol[:],
                                     b0_sb[0:1, wg * 384:(wg + 1) * 384],
                                     start=False, stop=True)
                    ldst = _ap(hnp1[:], bl * C * W + wg * 6, [[1, 6], [W, C]])
                    sin = p2[:].rearrange("p (a b) -> p a b", a=6)
                    nc.scalar.activation(ldst, sin, AF.Lrelu)

                for bl in range(BP):
                    _attn(bl)
                    for wg in range(8):
                        ht_ = _conv0(bl, wg)
                        _agg0(bl, wg, ht_)

              # ================= block1 (agg-first) =================
              hcp2 = bk.tile([128, CPF], BF16, tag="hcpA", name="hcp2")
              with tc.tile_pool(name="bkc1", bufs=1) as bkc1, \
                 tc.tile_pool(name="b1wk", bufs=2) as b1w, \
                 tc.tile_pool(name="b1ps", bufs=3, space="PSUM") as b1ps, \
                 tc.tile_pool(name="b1ps2", bufs=3, space="PSUM") as b1ps2:
                hcp1 = bkc1.tile([128, CPF], BF16, tag="hcp1")

                for off in (0, 3 + W):
                    zdst = _ap(hcp1[:], off, [[WP, BP * N], [1, 3]])
                    zsrc = _ap(zb_sb[:], 0, [[0, BP * N], [1, 3]])
                    nc.vector.tensor_copy(zdst, zsrc)

                for bl in range(BP):
                    y1 = b1w.tile([128, 3072], BF16, tag="y1")
                    for s in range(6):
                        pa = b1ps.tile([128, 512], F32, tag="pa")
                        nc.tensor.matmul(pa[:], aggw[bl][:],
                                         hnp1[:, bl * 3072 + s * 512: bl * 3072 + (s + 1) * 512],
                                         start=True, stop=True)
                        nc.scalar.activation(y1[:, s * 512:(s + 1) * 512], pa[:],
                                             AF.Identity)
                    # flip NP->CP via DRAM scratch
                    for pl in range(2):
                        src = _ap(y1[pl * 64:(pl + 1) * 64, :], 0, [[48, C], [1, 48]])
                        dst = bass.AP(
                            tensor=scr[:].tensor,
                            offset=scr[:].offset + (pl * 64) * (BP * N * W) + bl * N * W,
                            ap=[[48, 64], [BP * N * W, 64], [1, 48]])
                        nc.gpsimd.dma_start(dst, src)
                    rb_src = bass.AP(
                        tensor=scr[:].tensor,
                        offset=scr[:].offset + bl * N * W,
                        ap=[[BP * N * W, 128], [48, 64], [1, 48]])
                    rb_dst = _ap(hcp1[:], bl * N * WP + 3, [[WP, N], [1, 48]])
                    nc.sync.dma_start(rb_dst, rb_src)

                for bl in range(BP):
                    for wg in range(6):
                        base = bl * N * WP + 3 + wg * 8
                        pc = b1ps2.tile([128, N, 8], F32, tag="pc")
                        for tt in range(5):
                            rhs = _ap(hcp1[:], base + (tt - 2), [[WP, N], [1, 8]])
                            nc.tensor.matmul(
                                pc[:], mw_sb[:, (TOFF[1] + tt) * 128:(TOFF[1] + tt + 1) * 128],
                                rhs, start=(tt == 0), stop=(tt == 4))
                        gsl = _ap(gbc1_sb[:], wg * 8, [[0, N], [1, 8]])
                        nc.vector.tensor_add(pc[:], pc[:], gsl)
                        ldst = _ap(hcp2[:], base, [[WP, N], [1, 8]])
                        nc.scalar.activation(ldst, pc[:], AF.Lrelu)

            # ================= block2 (conv-first) =================
            with tc.tile_pool(name="b2wk", bufs=3) as b2w, \
                 tc.tile_pool(name="stg", bufs=2) as stg, \
                 tc.tile_pool(name="c2ps", bufs=2, space="PSUM") as c2ps, \
                 tc.tile_pool(name="c2ps2", bufs=2, space="PSUM") as c2ps2, \
                 tc.tile_pool(name="c2ps3", bufs=2, space="PSUM") as c2ps3:

                for off in (0, 3 + W):
                    zdst = _ap(hcp2[:], off, [[WP, BP * N], [1, 3]])
                    zsrc = _ap(zb_sb[:], 0, [[0, BP * N], [1, 3]])
                    nc.vector.tensor_copy(zdst, zsrc)

                for bl in range(BP):
                    hst = stg.tile([128, C * W], BF16, tag="hst")
                    for wg in range(8):
                        base = bl * N * WP + 3 + wg * 6
                        p1 = c2ps.tile([128, N, 6], F32, tag="p1")
                        for tt in range(7):
                            rhs = _ap(hcp2[:], base + (tt - 3), [[WP, N], [1, 6]])
                            nc.tensor.matmul(
                                p1[:], mw_sb[:, (TOFF[2] + tt) * 128:(TOFF[2] + tt + 1) * 128],
                                rhs, start=(tt == 0), stop=(tt == 6))
                        s1 = b2w.tile([128, N, 6], BF16, tag="s1")
                        nc.vector.tensor_copy(s1[:], p1[:])
                        p1t = c2ps2.tile([128, 6, C], BF16, tag="p1t")
                        for pl in range(2):
                            for wl in range(6):
                                sl = s1[pl * 64:pl * 64 + 64, :, wl]
                                tin = bass.AP(tensor=sl.tensor, offset=sl.offset,
                                              ap=[list(sl.ap[0]), [6, N]])
                                nc.tensor.transpose(p1t[pl * 64:pl * 64 + 64, wl, :],
                                                    tin, id2_sb[pl * 64:pl * 64 + 64, :])
                        hnt = b2w.tile([128, 6, C], BF16, tag="hnt")
                        nc.scalar.activation(hnt[:], p1t[:], AF.Identity)
                        p2 = c2ps3.tile([128, 6 * C], F32, tag="p2")
                        nc.tensor.matmul(p2[:], aggw[bl][:],
                                         hnt[:].rearrange("p a b -> p (a b)"),
                                         start=True, stop=False)
                        nc.tensor.matmul(p2[:], onescol[:],
                                         b2_sb[0:1, wg * 384:(wg + 1) * 384],
                                         start=False, stop=True)
                        nc.scalar.activation(hst[:, wg * 384:(wg + 1) * 384],
                                             p2[:], AF.Lrelu)
                    for pl in range(2):
                        sl = hst[pl * 64:pl * 64 + 64, :]
                        for jj in range(8):
                            asrc = bass.AP(tensor=sl.tensor,
                                           offset=sl.offset + jj * 6 * C,
                                           ap=[list(sl.ap[0])] + [[C, 6], [1, C]])
                            adst = bass.AP(
                                tensor=a2a_in[:].tensor,
                                offset=a2a_in[:].offset + (jj * BL + pl * BP + bl) * KCH,
                                ap=[[C, N], [N * C, 6], [1, C]])
                            nc.sync.dma_start(adst, asrc)

            # ================= A2A + final linear + AR =================
            nc.gpsimd.collective_compute(
                "AllToAll", ALU.bypass,
                replica_groups=[list(range(NCORES))],
                ins=[a2a_in.opt()], outs=[a2a_out.opt()])

            with tc.tile_pool(name="fin", bufs=4) as fp, \
                 tc.tile_pool(name="fho", bufs=1) as fho, \
                 tc.tile_pool(name="fps", bufs=3, space="PSUM") as fps, \
                 tc.tile_pool(name="fpo", bufs=1, space="PSUM") as fpo:

                ho_sb = fho.tile([64, KCH], BF16, tag="ho")
                nc.sync.dma_start(ho_sb[:], a2a_out[:])

                pout = []
                for h in range(2):
                    po_t = fpo.tile([64, 384], F32, tag=f"pout{h}", name=f"pout{h}")
                    pout.append(po_t)

                for j in range(NWT):
                    wt = wt_tiles[j]
                    for kk in range(4):
                        kt_ = 4 * j + kk
                        pt = fps.tile([128, 64], BF16, tag="pt")
                        nc.tensor.transpose(pt[:], ho_sb[:, kt_ * 128:(kt_ + 1) * 128],
                                            id_sb[:])
                        ht = fp.tile([128, 64], BF16, tag="ht")
                        nc.scalar.activation(ht[:], pt[:], AF.Identity)
                        for hh in range(2):
                            nc.tensor.matmul(
                                pout[hh][:], ht[:],
                                wt[:, kk * RO + hh * 384: kk * RO + (hh + 1) * 384],
                                start=(kt_ == 0), stop=(kt_ == 4 * NWT - 1),
                                skip_group_check=True)
                    jj = j + WPRE
                    if jj < NWT:
                        _issue_wt(jj, nc.sync if jj % 2 == 0 else nc.gpsimd)

                oo = fp.tile([64, RO], F32, tag="oo")
                for hh in range(2):
                    nc.scalar.activation(oo[:, hh * 384:(hh + 1) * 384], pout[hh][:],
                                         AF.Identity)
                nc.sync.dma_start(ar_in[:], oo[:])
                nc.gpsimd.collective_compute(
                    "AllReduce", ALU.add,
                    replica_groups=[list(range(NCORES))],
                    ins=[ar_in.opt()], outs=[ar_out.opt()])
                oo2 = fp.tile([64, RO], F32, tag="oo2")
                nc.sync.dma_start(oo2[:], ar_out[:])
                nc.sync.dma_start(out[:], oo2[:])

    nc.compile()
    return nc


_NC = None


def _host_prep(inputs):
    f32 = np.float32
    bf16 = ml_dtypes.bfloat16
    x = np.asarray(inputs["x"], f32)
    w_ih = np.asarray(inputs["gru_w_ih"], f32)
    w_hh = np.asarray(inputs["gru_w_hh"], f32)
    b_ih = np.asarray(inputs["gru_b_ih"], f32)
    b_hh = np.asarray(inputs["gru_b_hh"], f32)

    shared = {
        "state0": np.zeros((64, SEQ), f32),
        "gru_hh_rz": np.ascontiguousarray(w_hh[0:128].T),
        "gru_ih_rz": np.ascontiguousarray(w_ih[0:128, 0][None, :]).astype(bf16),
        "gru_hh_n": np.ascontiguousarray(w_hh[128:192].T),
        "gru_ih_n": np.ascontiguousarray(w_ih[128:192, 0][None, :]).astype(bf16),
        "bias_r": (b_ih + b_hh)[0:64, None].copy(),
        "bias_z": (b_ih + b_hh)[64:128, None].copy(),
        "bias_ihn": b_ih[128:192, None].copy(),
        "bhhn_row": b_hh[128:192][None, :].astype(bf16),
        "onesrow": np.ones((1, SEQ)).astype(bf16),
        "wqT": np.ascontiguousarray(np.asarray(inputs["wq_w"], f32).T),
        "wkT": np.ascontiguousarray(np.asarray(inputs["wk_w"], f32).T),
        "wqb": np.asarray(inputs["wq_b"], f32)[:, None].copy(),
        "wkb": np.asarray(inputs["wk_b"], f32)[:, None].copy(),
        "ones128": np.ones((128, 1), f32),
        "idf128": np.vstack([np.eye(64), np.eye(64)]).astype(f32),
        "onescol_b": np.ones((1, 128)).astype(bf16),
        "identb": np.eye(64).astype(bf16),
        "identb2": np.vstack([np.eye(64), np.eye(64)]).astype(bf16),
        "zerosb": np.zeros((128, 128)).astype(bf16),
    }

    mwbd = np.zeros((15, 128, 128), f32)
    grows = []
    for i in range(3):
        gw_ = np.asarray(inputs[f"gcn_w{i}"], f32)
        gb = np.asarray(inputs[f"gcn_b{i}"], f32)
        cw = np.asarray(inputs[f"conv_w{i}"], f32)
        cb = np.asarray(inputs[f"conv_b{i}"], f32)
        k = KS[i]
        pad = k // 2
        for t in range(k):
            q = (cw[:, :, t] @ gw_).T
            mwbd[TOFF[i] + t, 0:64, 0:64] = q
            mwbd[TOFF[i] + t, 64:128, 64:128] = q
        cgt = np.einsum("oit,i->ot", cw, gb)
        g_ = np.zeros((C, W), f32)
        for w in range(W):
            for t in range(k):
                if 0 <= w + t - pad < W:
                    g_[:, w] += cgt[:, t]
        g_ += cb[:, None]
        grows.append(g_)

    shared["mwbd"] = mwbd.astype(bf16)
    shared["b0row"] = np.ascontiguousarray(grows[0].T.reshape(1, C * W)).astype(bf16)
    shared["gbc1"] = np.vstack([grows[1], grows[1]]).astype(f32)
    shared["b2row"] = np.ascontiguousarray(grows[2].T.reshape(1, C * W)).astype(bf16)

    emb_w = np.asarray(inputs["emb_w"], f32)
    emb_b = np.asarray(inputs["emb_b"], f32)
    lout_w = np.asarray(inputs["lout_w"], f32)

    in_maps = []
    for c_ in range(NCORES):
        xc = x[c_ * BL:(c_ + 1) * BL]
        m = dict(shared)
        m["x1"] = np.ascontiguousarray(
            xc.transpose(1, 0, 2).reshape(1, W * SEQ)).astype(bf16)
        h0 = xc[..., None] * emb_w + emb_b
        hcp_h = np.zeros((2, 64, BP, N, WP), f32)
        hsrc = h0.reshape(2, BP, W, N, C).transpose(0, 4, 1, 3, 2)
        hcp_h[:, :, :, :, 3:3 + W] = hsrc
        m["h0cp"] = np.ascontiguousarray(hcp_h.reshape(128, CPF)).astype(bf16)
        lw = lout_w[:, c_ * KCH:(c_ + 1) * KCH]
        m["lwT"] = np.ascontiguousarray(lw.T).astype(bf16)
        in_maps.append(m)
    return in_maps


def kernel_with_stats(**inputs):
    global _NC
    if _NC is None:
        _NC = _build()
    in_maps = _host_prep(inputs)
    trace = os.environ.get("KERNEL_TRACE", "") == "1"
    res = run_bass_kernel_spmd(_NC, in_maps, core_ids=list(range(NCORES)), trace=trace)
    out = res.results[0]["out"] + np.asarray(inputs["lout_b"], np.float32)[None, :]
    return out.reshape(B, HOR, N).astype(np.float32), res


def kernel(**inputs):
    o, _ = kernel_with_stats(**inputs)
    return o
